# revision 22
# baseline (speedup 1.0000x reference)
"""Soft-KNN Bass/Tile kernel for Trainium2 (8 NeuronCores, axon/PJRT).

Strategy (v2)
-------------
- Shard train set (50000 rows) across 8 cores, 6250 rows each, sorted by
  label host-side. Host precomputes transposed fp16/fp8 operand tensors,
  norm-ladder rows, and a per-column label plane, so the device does no
  transposes and no norm computation.
- z = 2*x.y + (512 - ||y||^2) computed per (query-tile, 512-col chunk) as:
    1 bf16 ladder matmul (2-row ync residual pair)
  + 4 fp16 matmuls (hi x hi, K=128 each)
  + 4 e4m3 cross-term matmuls folded into 2+2 DoubleRow matmuls (K=256):
      e4m3(64*xl).e4m3(yh/64) + e4m3(xh/64).e4m3(64*yl)
    (symmetric power-of-2 scaling cancels exactly in the product).
  Total ~3.6k PE cycles per chunk vs ~9k for the f32r 3-product split.
- Selection: per-query-row label packing: low 7 mantissa bits of each z
  are replaced by the column's class label via one fused DVE
  scalar_tensor_tensor on the low-byte lane: (z & 0x80) | lab.  A single
  full-row vector.max then yields the top-8 packed candidates per core;
  labels travel inside the values, so no indices, no merge network.
- One AllGather of [2048, 8] fp32 packed candidates. Each core owns 2
  query tiles; the exact global top-16 of its 64 gathered candidates is
  extracted with max8 + match_replace + max8, labels unpacked with an
  i32 AND, then softmax(-sqrt(xn + 512 - z)) and a 16-round is_equal
  scatter-add vote into 100 classes.
"""

import os
import numpy as np

import concourse.bass as bass
import concourse.bacc as bacc
import concourse.mybir as mybir
import concourse.tile as tile
from concourse import bass_utils

F32 = mybir.dt.float32
F16 = mybir.dt.float16
BF16 = mybir.dt.bfloat16
F8E4 = mybir.dt.float8e4
U16 = mybir.dt.uint16
U8 = mybir.dt.uint8
I32 = mybir.dt.int32
AL = mybir.AluOpType
AF = mybir.ActivationFunctionType
DR = mybir.MatmulPerfMode.DoubleRow

NCORES = 8
B = 2048                 # queries
D = 512                  # feature dim
NSHARD = 6250            # train rows per core
COLS = 6272              # padded columns (12*512 + 128)
CHUNKS = [512] * 12 + [128]
NCHUNK = len(CHUNKS)     # 13
QTILES = B // 128        # 16
NCLASS = 100
K = 16
NG = NCORES * 8          # 64 gathered candidates per query
NEG = -3.0e38            # match_replace marker
S = 64.0                 # symmetric fp8 cross-term scale

STAGE = int(os.environ.get("KNN_STAGE", "3"))
DEBUG = int(os.environ.get("KNN_DEBUG", "0"))
NOPACK = int(os.environ.get("KNN_NOPACK", "0"))
NODR = int(os.environ.get("KNN_NODR", "0"))
DRINT = int(os.environ.get("KNN_DRINT", "0"))


def _coff(c):
    return sum(CHUNKS[:c])


def build():
    nc = bacc.Bacc("TRN2", target_bir_lowering=False, num_devices=NCORES)

    xh16_d = nc.dram_tensor("xh16", [4, 128, B], F16, kind="ExternalInput")
    xl8_d = nc.dram_tensor("xl8", [4, 128, B], F8E4, kind="ExternalInput")
    xh8_d = nc.dram_tensor("xh8", [4, 128, B], F8E4, kind="ExternalInput")
    trh16_d = nc.dram_tensor("trh16", [4, 128, COLS], F16, kind="ExternalInput")
    trh8_d = nc.dram_tensor("trh8", [128, 4 * COLS], F8E4,
                            kind="ExternalInput")
    trl8_d = nc.dram_tensor("trl8", [128, 4 * COLS], F8E4,
                            kind="ExternalInput")
    ync_d = nc.dram_tensor("ync", [2, COLS], BF16, kind="ExternalInput")
    lab_d = nc.dram_tensor("lab", [128, COLS], U16, kind="ExternalInput")
    xn_d = nc.dram_tensor("xn512", [128, QTILES], F32, kind="ExternalInput")
    out_d = nc.dram_tensor("out", [2 * 128, NCLASS], F32, kind="ExternalOutput")

    ag_in = nc.dram_tensor("ag_in", [B, 8], F32)
    ag_out = nc.dram_tensor("ag_out", [NCORES * B, 8], F32,
                            addr_space="Shared")
    if DEBUG:
        dbgz_d = nc.dram_tensor("dbgz", [128, COLS], F32,
                                kind="ExternalOutput")
        dbgp_d = nc.dram_tensor("dbgp", [128, COLS], F32,
                                kind="ExternalOutput")
        dbgc_d = nc.dram_tensor("dbgc", [128, 8], F32,
                                kind="ExternalOutput")

    with tile.TileContext(nc) as tc:
        with tc.tile_pool(name="res", bufs=1) as res, \
             tc.tile_pool(name="zps", bufs=8, space="PSUM") as zps:

            # ------------- resident tensors -------------
            xh16 = [res.tile([128, B], F16, name=f"xh16_{k}", tag=f"xh16_{k}")
                    for k in range(4)]
            xl8 = res.tile([128, 4 * B], F8E4, name="xl8", tag="xl8")
            xh8 = res.tile([128, 4 * B], F8E4, name="xh8", tag="xh8")
            trh16 = [res.tile([128, COLS], F16, name=f"trh16_{k}",
                              tag=f"trh16_{k}") for k in range(4)]
            trh8 = res.tile([128, 4 * COLS], F8E4, name="trh8", tag="trh8")
            trl8 = res.tile([128, 4 * COLS], F8E4, name="trl8", tag="trl8")
            ync = res.tile([2, COLS], BF16, name="ync", tag="ync")
            lab = res.tile([128, COLS], U16, name="lab", tag="lab")
            xn_s = res.tile([128, QTILES], F32, name="xn_s", tag="xn_s")
            ones2 = res.tile([2, 128], BF16, name="ones2", tag="ones2")
            cio_f = res.tile([128, NCLASS], F32, name="cio_f", tag="cio_f")
            m16 = res.tile([128, 1], U16, name="m16", tag="m16")
            m127 = res.tile([128, 1], I32, name="m127", tag="m127")

            nc.vector.memset(ones2[:], 1.0)
            nc.vector.memset(m16[:], 0xFF80)
            nc.vector.memset(m127[:], 127)
            with tc.tile_pool(name="sup", bufs=1) as sup:
                cio_i = sup.tile([128, NCLASS], I32, tag="cioi")
                nc.gpsimd.iota(cio_i[:], pattern=[[1, NCLASS]],
                               channel_multiplier=0)
                nc.vector.tensor_copy(cio_f[:], cio_i[:])

            # ------------- input DMAs (first-needed first) -------------
            H = COLS // 2
            nc.sync.dma_start(ync[:], ync_d[:])
            nc.sync.dma_start(xn_s[:], xn_d[:])
            for k in range(4):
                nc.sync.dma_start(xh16[k][:, 0:128], xh16_d[k, :, 0:128])
                nc.sync.dma_start(xl8[:, k * B:k * B + 128],
                                  xl8_d[k, :, 0:128])
                nc.sync.dma_start(xh8[:, k * B:k * B + 128],
                                  xh8_d[k, :, 0:128])
            for k in range(4):
                nc.sync.dma_start(trh16[k][:, 0:H], trh16_d[k, :, 0:H])
                nc.sync.dma_start(trh8[:, k * COLS:k * COLS + H],
                                  trh8_d[:, k * COLS:k * COLS + H])
                nc.sync.dma_start(trl8[:, k * COLS:k * COLS + H],
                                  trl8_d[:, k * COLS:k * COLS + H])
            nc.sync.dma_start(lab[:], lab_d[:])
            for k in range(4):
                nc.sync.dma_start(xh16[k][:, 128:B], xh16_d[k, :, 128:B])
                nc.sync.dma_start(xl8[:, k * B + 128:(k + 1) * B],
                                  xl8_d[k, :, 128:B])
                nc.sync.dma_start(xh8[:, k * B + 128:(k + 1) * B],
                                  xh8_d[k, :, 128:B])
            for k in range(4):
                nc.sync.dma_start(trh16[k][:, H:COLS], trh16_d[k, :, H:COLS])
                nc.sync.dma_start(trh8[:, k * COLS + H:(k + 1) * COLS],
                                  trh8_d[:, k * COLS + H:(k + 1) * COLS])
                nc.sync.dma_start(trl8[:, k * COLS + H:(k + 1) * COLS],
                                  trl8_d[:, k * COLS + H:(k + 1) * COLS])

            xl8_v = xl8[:].rearrange("p (s n) -> p s n", s=4)
            xh8_v = xh8[:].rearrange("p (s n) -> p s n", s=4)
            if DRINT:
                # interleaved pair-innermost: [p][j][n][2]
                trh8_v = trh8[:].rearrange("p (j n two) -> p j n two",
                                           j=2, two=2)
                trl8_v = trl8[:].rearrange("p (j n two) -> p j n two",
                                           j=2, two=2)
            else:
                trh8_v = trh8[:].rearrange("p (s n) -> p s n", s=4)
                trl8_v = trl8[:].rearrange("p (s n) -> p s n", s=4)

            # ------------- main loop -------------
            with tc.tile_pool(name="zbuf", bufs=2) as zbuf, \
                 tc.tile_pool(name="small", bufs=2) as small:

                def global_phase(l, ag_t):
                    gv = small.tile([128, NG], F32, name=f"gv{l}", tag="gv",
                                    bufs=1)
                    qrow = (pid_sp + 8 * l) * 128
                    for c2 in range(NCORES):
                        nc.sync.dma_start(
                            gv[:, c2 * 8:(c2 + 1) * 8],
                            ag_t[bass.ds(c2 * B + qrow, 128), 0:8])
                    t8a = small.tile([128, 8], F32, name=f"t8a{l}", tag="t8a")
                    t8b = small.tile([128, 8], F32, name=f"t8b{l}", tag="t8b")
                    gv2 = small.tile([128, NG], F32, name=f"gv2{l}", tag="gv2",
                                     bufs=1)
                    nc.vector.max(t8a[:], gv[:])
                    nc.vector.match_replace(gv2[:], t8a[:], gv[:], NEG)
                    nc.vector.max(t8b[:], gv2[:])
                    v16 = small.tile([128, K], F32, name=f"v16{l}", tag="v16")
                    nc.vector.tensor_copy(v16[:, 0:8], t8a[:])
                    nc.vector.tensor_copy(v16[:, 8:16], t8b[:])
                    labi = small.tile([128, K], I32, name=f"labi{l}",
                                      tag="labi")
                    nc.vector.tensor_scalar(
                        out=labi[:], in0=v16[:].bitcast(I32),
                        scalar1=m127[:, 0:1], scalar2=None,
                        op0=AL.bitwise_and)
                    lab16 = small.tile([128, K], F32, name=f"lab16{l}",
                                       tag="lab16")
                    nc.vector.tensor_copy(lab16[:], labi[:])
                    xn_col = small.tile([128, 1], F32, name=f"xnc{l}",
                                        tag="xncol")
                    nc.sync.dma_start(xn_col[:],
                                      xn_d[:, bass.ds(pid_sp + 8 * l, 1)])
                    dsq = small.tile([128, K], F32, name=f"dsq{l}", tag="dsq")
                    nc.scalar.activation(dsq[:], v16[:], AF.Sqrt, scale=-1.0,
                                         bias=xn_col[:, 0:1])
                    ew = small.tile([128, K], F32, name=f"ew{l}", tag="ew")
                    zsum = small.tile([128, 1], F32, name=f"zs{l}", tag="zs")
                    nc.scalar.activation(ew[:], dsq[:], AF.Exp, scale=-1.0,
                                         accum_out=zsum[:, 0:1])
                    rz = small.tile([128, 1], F32, name=f"rz{l}", tag="rz")
                    nc.vector.reciprocal(rz[:], zsum[:])
                    wt = small.tile([128, K], F32, name=f"wt{l}", tag="wt")
                    nc.vector.tensor_scalar(out=wt[:], in0=ew[:],
                                            scalar1=rz[:, 0:1], scalar2=None,
                                            op0=AL.mult)
                    vote = small.tile([128, NCLASS], F32, name=f"vote{l}",
                                      tag="vote", bufs=1)
                    tmp = small.tile([128, NCLASS], F32, name=f"vtmp{l}",
                                     tag="vtmp", bufs=1)
                    nc.vector.memset(vote[:], 0.0)
                    for r in range(K):
                        nc.vector.tensor_scalar(out=tmp[:], in0=cio_f[:],
                                                scalar1=lab16[:, r:r + 1],
                                                scalar2=wt[:, r:r + 1],
                                                op0=AL.is_equal, op1=AL.mult)
                        nc.vector.tensor_tensor(out=vote[:], in0=vote[:],
                                                in1=tmp[:], op=AL.add)
                    nc.sync.dma_start(out_d[l * 128:(l + 1) * 128, :], vote[:])

                pid_sp = nc.sync.partition_id()
                for qt in range(QTILES if STAGE >= 1 else 0):
                    qs = qt * 128
                    z = zbuf.tile([128, COLS], F32, name=f"z{qt}", tag="z")
                    for grp in (range(0, 8), range(8, 13)):
                        pss = {}
                        for c in grp:
                            pss[c] = zps.tile([128, 512], F32,
                                              name=f"ps_{qt}_{c}", tag="ps")
                        for k in range(4):
                            for c in grp:
                                cw = CHUNKS[c]
                                co = _coff(c)
                                nc.tensor.matmul(
                                    pss[c][:, :cw],
                                    xh16[k][:, qs:qs + 128],
                                    trh16[k][:, co:co + cw],
                                    start=(k == 0), stop=False)
                        if NODR:
                            for j in range(4):
                                for c in grp:
                                    cw = CHUNKS[c]
                                    co = _coff(c)
                                    xv = (xl8_v, xh8_v)[j // 2]
                                    tv = (trh8_v, trl8_v)[j // 2]
                                    for s in (2 * (j % 2), 2 * (j % 2) + 1):
                                        nc.tensor.matmul(
                                            pss[c][:, :cw],
                                            xv[:, s, qs:qs + 128],
                                            tv[:, s, co:co + cw],
                                            start=False, stop=False)
                        else:
                            for half in range(2):
                                xv = (xl8_v, xh8_v)[half]
                                tv = (trh8_v, trl8_v)[half]
                                for j in range(2):
                                    for c in grp:
                                        cw = CHUNKS[c]
                                        co = _coff(c)
                                        if DRINT:
                                            rhs = tv[:, j, co:co + cw, :]
                                        else:
                                            rhs = tv[:, 2 * j:2 * j + 2,
                                                     co:co + cw]
                                        nc.tensor.matmul(
                                            pss[c][:, :cw],
                                            xv[:, 2 * j:2 * j + 2,
                                               qs:qs + 128],
                                            rhs,
                                            start=False, stop=False,
                                            perf_mode=DR)
                        for c in grp:
                            cw = CHUNKS[c]
                            co = _coff(c)
                            nc.tensor.matmul(pss[c][:, :cw], ones2[:],
                                             ync[:, co:co + cw],
                                             start=False, stop=True)
                        for c in grp:
                            cw = CHUNKS[c]
                            co = _coff(c)
                            nc.scalar.copy(z[:, co:co + cw], pss[c][:, :cw])

                    if DEBUG and qt == 0:
                        nc.sync.dma_start(dbgz_d[:], z[:])
                    HB = 8 * 512   # group-1 column boundary (4096)
                    cand16 = small.tile([128, 16], F32, name=f"c16_{qt}",
                                        tag="cand16")
                    for h, (c0, c1) in enumerate(((0, HB), (HB, COLS))):
                        if not NOPACK:
                            zw = (z[:, c0:c1].bitcast(U16)
                                  .rearrange("p (m two) -> p m two", two=2)
                                  [:, :, 0:1])
                            nc.vector.scalar_tensor_tensor(
                                out=zw, in0=zw, scalar=m16[:, 0:1],
                                op0=AL.bitwise_and,
                                in1=lab[:, c0:c1].rearrange(
                                    "p (m one) -> p m one", one=1),
                                op1=AL.bitwise_or)
                        nc.vector.max(cand16[:, 8 * h:8 * h + 8],
                                      z[:, c0:c1])
                    cand = small.tile([128, 8], F32, name=f"cand{qt}",
                                      tag="cand")
                    nc.vector.max(cand[:], cand16[:])
                    nc.sync.dma_start(ag_in[qs:qs + 128, 0:8], cand[:])
                    if DEBUG and qt == 0:
                        nc.sync.dma_start(dbgp_d[:], z[:])
                        nc.sync.dma_start(dbgc_d[:], cand[:])

                if STAGE >= 3:
                    nc.gpsimd.collective_compute(
                        "AllGather", AL.bypass,
                        replica_groups=[list(range(NCORES))],
                        ins=[ag_in[:].opt()], outs=[ag_out[:].opt()])
                    global_phase(0, ag_out)
                    global_phase(1, ag_out)

    nc.finalize()
    return nc


_NC_CACHE = None

LDWOPT = int(os.environ.get("KNN_LDWOPT", "0"))
if LDWOPT:
    from concourse import bass_utils as _bu
    _orig_run_command = _bu.run_command

    def _patched_run_command(cmd, *a, **kw):
        if isinstance(cmd, list):
            cmd = ["--enable-ldw-opt=true" if c == "--enable-ldw-opt=false"
                   else c for c in cmd]
        return _orig_run_command(cmd, *a, **kw)

    _bu.run_command = _patched_run_command


def _host_prep(x, tf, tl):
    """Per-core host preprocessing -> list of input dicts."""
    import ml_dtypes
    bf16 = ml_dtypes.bfloat16
    e4m3 = ml_dtypes.float8_e4m3

    x = np.ascontiguousarray(np.asarray(x, dtype=np.float32))
    tf = np.ascontiguousarray(np.asarray(tf, dtype=np.float32))
    tl = np.asarray(tl)

    xs = 2.0 * x                                     # [B, D]
    xh16 = xs.astype(np.float16)
    xh16f = xh16.astype(np.float32)
    xl = xs - xh16f
    xl8 = (xl * S).astype(e4m3)
    xh8 = (xh16f / S).astype(e4m3)

    def to_k128(a):  # [B, D] -> [4, 128, B] (transposed k-split)
        return np.ascontiguousarray(a.T.reshape(4, 128, -1))

    xh16_t = to_k128(xh16)
    xl8_t = to_k128(xl8)
    xh8_t = to_k128(xh8)

    xn = (x.astype(np.float64) ** 2).sum(1).astype(np.float32) + 512.0
    xn_t = np.ascontiguousarray(xn.reshape(QTILES, 128).T)

    in_maps = []
    for c in range(NCORES):
        sl = slice(c * NSHARD, (c + 1) * NSHARD)
        labs = np.asarray(tl[sl], dtype=np.int64)
        feats = tf[sl]
        perm = np.argsort(labs, kind="stable")
        feats_s = np.ascontiguousarray(feats[perm])
        labs_s = labs[perm]

        yh16 = feats_s.astype(np.float16)
        yh16f = yh16.astype(np.float32)
        yl = feats_s - yh16f

        trh16 = np.zeros((4, 128, COLS), np.float16)
        trh16[:, :, :NSHARD] = yh16.T.reshape(4, 128, -1)
        h8 = np.zeros((4, 128, COLS), e4m3)
        l8 = np.zeros((4, 128, COLS), e4m3)
        h8[:, :, :NSHARD] = (yh16f / S).astype(e4m3).T.reshape(4, 128, -1)
        l8[:, :, :NSHARD] = (yl * S).astype(e4m3).T.reshape(4, 128, -1)
        if DRINT:
            # out[p, j, n, s] = a[2j+s, p, n] -> [128, 4*COLS]
            def inter(a):
                st = a.reshape(2, 2, 128, COLS)
                st = np.transpose(st, (2, 0, 3, 1))
                return np.ascontiguousarray(st.reshape(128, 4 * COLS))
            trh8 = inter(h8)
            trl8 = inter(l8)
        else:
            trh8 = np.ascontiguousarray(
                np.moveaxis(h8, 0, 1).reshape(128, 4 * COLS))
            trl8 = np.ascontiguousarray(
                np.moveaxis(l8, 0, 1).reshape(128, 4 * COLS))

        yn = (feats_s.astype(np.float64) ** 2).sum(1).astype(np.float32)
        t = 512.0 - yn
        y1 = t.astype(bf16).astype(np.float32)
        y2 = (t - y1).astype(bf16)
        ync = np.zeros((2, COLS), bf16)
        ync[0, :NSHARD] = y1.astype(bf16)
        ync[1, :NSHARD] = y2
        ync[0, NSHARD:] = np.float32(-30000.0)

        labp = np.zeros(COLS, np.uint16)
        labp[:NSHARD] = labs_s.astype(np.uint16)
        lab_b = np.ascontiguousarray(np.broadcast_to(labp, (128, COLS)))

        in_maps.append({
            "xh16": xh16_t, "xl8": xl8_t, "xh8": xh8_t,
            "trh16": trh16, "trh8": trh8, "trl8": trl8,
            "ync": ync, "lab": lab_b, "xn512": xn_t,
        })
    return in_maps


def kernel(x, train_features, train_labels, **run_kwargs):
    global _NC_CACHE
    in_maps = _host_prep(x, train_features, train_labels)
    if _NC_CACHE is None:
        _NC_CACHE = build()
    res = bass_utils.run_bass_kernel_spmd(
        _NC_CACHE, in_maps, core_ids=list(range(NCORES)), **run_kwargs)
    global LAST_RESULTS
    LAST_RESULTS = res
    out = np.empty((B, NCLASS), np.float32)
    for q in range(QTILES):
        l = q // NCORES
        out[q * 128:(q + 1) * 128] = (
            res.results[q % NCORES]["out"][l * 128:(l + 1) * 128])
    return out.astype(np.float32)


LAST_RESULTS = None


# revision 24
# speedup vs baseline: 1.0002x; 1.0002x over previous
"""Soft-KNN Bass/Tile kernel for Trainium2 (8 NeuronCores, axon/PJRT).

Strategy (v2)
-------------
- Shard train set (50000 rows) across 8 cores, 6250 rows each, sorted by
  label host-side. Host precomputes transposed fp16/fp8 operand tensors,
  norm-ladder rows, and a per-column label plane, so the device does no
  transposes and no norm computation.
- z = 2*x.y + (512 - ||y||^2) computed per (query-tile, 512-col chunk) as:
    1 bf16 ladder matmul (2-row ync residual pair)
  + 4 fp16 matmuls (hi x hi, K=128 each)
  + 4 e4m3 cross-term matmuls folded into 2+2 DoubleRow matmuls (K=256):
      e4m3(64*xl).e4m3(yh/64) + e4m3(xh/64).e4m3(64*yl)
    (symmetric power-of-2 scaling cancels exactly in the product).
  Total ~3.6k PE cycles per chunk vs ~9k for the f32r 3-product split.
- Selection: per-query-row label packing: low 7 mantissa bits of each z
  are replaced by the column's class label via one fused DVE
  scalar_tensor_tensor on the low-u16 lane: (z.lo16 & 0xFF80) | lab,
  done per half-qtile so it pipelines behind the PSUM drains.  vector.max
  per half + a 16-wide combine yields the top-8 packed candidates per
  core; labels travel inside the values, so no indices, no merge network.
- One AllGather of [2048, 8] fp32 packed candidates. Each core owns 2
  query tiles; the exact global top-16 of its 64 gathered candidates is
  extracted with max8 + match_replace + max8, labels unpacked with an
  i32 AND, then softmax(-sqrt(xn + 512 - z)) and a 16-round is_equal
  scatter-add vote into 100 classes.
"""

import os
import numpy as np

import concourse.bass as bass
import concourse.bacc as bacc
import concourse.mybir as mybir
import concourse.tile as tile
from concourse import bass_utils

F32 = mybir.dt.float32
F16 = mybir.dt.float16
BF16 = mybir.dt.bfloat16
F8E4 = mybir.dt.float8e4
U16 = mybir.dt.uint16
U8 = mybir.dt.uint8
I32 = mybir.dt.int32
AL = mybir.AluOpType
AF = mybir.ActivationFunctionType
DR = mybir.MatmulPerfMode.DoubleRow

NCORES = 8
B = 2048                 # queries
D = 512                  # feature dim
NSHARD = 6250            # train rows per core
COLS = 6272              # padded columns (12*512 + 128)
CHUNKS = [512] * 12 + [128]
NCHUNK = len(CHUNKS)     # 13
QTILES = B // 128        # 16
NCLASS = 100
K = 16
NG = NCORES * 8          # 64 gathered candidates per query
NEG = -3.0e38            # match_replace marker
S = 64.0                 # symmetric fp8 cross-term scale

STAGE = int(os.environ.get("KNN_STAGE", "3"))
DEBUG = int(os.environ.get("KNN_DEBUG", "0"))
NOPACK = int(os.environ.get("KNN_NOPACK", "0"))
NODR = int(os.environ.get("KNN_NODR", "0"))
DRINT = int(os.environ.get("KNN_DRINT", "0"))


def _coff(c):
    return sum(CHUNKS[:c])


def build():
    nc = bacc.Bacc("TRN2", target_bir_lowering=False, num_devices=NCORES)

    xh16_d = nc.dram_tensor("xh16", [4, 128, B], F16, kind="ExternalInput")
    xl8_d = nc.dram_tensor("xl8", [4, 128, B], F8E4, kind="ExternalInput")
    xh8_d = nc.dram_tensor("xh8", [4, 128, B], F8E4, kind="ExternalInput")
    trh16_d = nc.dram_tensor("trh16", [4, 128, COLS], F16, kind="ExternalInput")
    trh8_d = nc.dram_tensor("trh8", [128, 4 * COLS], F8E4,
                            kind="ExternalInput")
    trl8_d = nc.dram_tensor("trl8", [128, 4 * COLS], F8E4,
                            kind="ExternalInput")
    ync_d = nc.dram_tensor("ync", [2, COLS], BF16, kind="ExternalInput")
    lab_d = nc.dram_tensor("lab", [128, COLS], U16, kind="ExternalInput")
    xn_d = nc.dram_tensor("xn512", [128, QTILES], F32, kind="ExternalInput")
    out_d = nc.dram_tensor("out", [2 * 128, NCLASS], F32, kind="ExternalOutput")

    ag_in = nc.dram_tensor("ag_in", [B, 8], F32)
    ag_out = nc.dram_tensor("ag_out", [NCORES * B, 8], F32,
                            addr_space="Shared")
    if DEBUG:
        dbgz_d = nc.dram_tensor("dbgz", [128, COLS], F32,
                                kind="ExternalOutput")
        dbgp_d = nc.dram_tensor("dbgp", [128, COLS], F32,
                                kind="ExternalOutput")
        dbgc_d = nc.dram_tensor("dbgc", [128, 8], F32,
                                kind="ExternalOutput")

    with tile.TileContext(nc) as tc:
        with tc.tile_pool(name="res", bufs=1) as res, \
             tc.tile_pool(name="zps", bufs=8, space="PSUM") as zps:

            # ------------- resident tensors -------------
            xh16 = [res.tile([128, B], F16, name=f"xh16_{k}", tag=f"xh16_{k}")
                    for k in range(4)]
            xl8 = res.tile([128, 4 * B], F8E4, name="xl8", tag="xl8")
            xh8 = res.tile([128, 4 * B], F8E4, name="xh8", tag="xh8")
            trh16 = [res.tile([128, COLS], F16, name=f"trh16_{k}",
                              tag=f"trh16_{k}") for k in range(4)]
            trh8 = res.tile([128, 4 * COLS], F8E4, name="trh8", tag="trh8")
            trl8 = res.tile([128, 4 * COLS], F8E4, name="trl8", tag="trl8")
            ync = res.tile([2, COLS], BF16, name="ync", tag="ync")
            lab = res.tile([128, COLS], U16, name="lab", tag="lab")
            xn_s = res.tile([128, QTILES], F32, name="xn_s", tag="xn_s")
            ones2 = res.tile([2, 128], BF16, name="ones2", tag="ones2")
            cio_f = res.tile([128, NCLASS], F32, name="cio_f", tag="cio_f")
            m16 = res.tile([128, 1], U16, name="m16", tag="m16")
            m127 = res.tile([128, 1], I32, name="m127", tag="m127")

            nc.vector.memset(ones2[:], 1.0)
            nc.vector.memset(m16[:], 0xFF80)
            nc.vector.memset(m127[:], 127)
            with tc.tile_pool(name="sup", bufs=1) as sup:
                cio_i = sup.tile([128, NCLASS], I32, tag="cioi")
                nc.gpsimd.iota(cio_i[:], pattern=[[1, NCLASS]],
                               channel_multiplier=0)
                nc.vector.tensor_copy(cio_f[:], cio_i[:])

            # ------------- input DMAs (first-needed first) -------------
            # progressive column slices so qtile-0 group-1 matmuls start ASAP
            nc.sync.dma_start(ync[:], ync_d[:])
            nc.sync.dma_start(xn_s[:], xn_d[:])
            for k in range(4):
                nc.sync.dma_start(xh16[k][:, 0:128], xh16_d[k, :, 0:128])
                nc.sync.dma_start(xl8[:, k * B:k * B + 128],
                                  xl8_d[k, :, 0:128])
                nc.sync.dma_start(xh8[:, k * B:k * B + 128],
                                  xh8_d[k, :, 0:128])
            for c0, c1 in ((0, 512), (512, 4096), (4096, COLS)):
                for k in range(4):
                    nc.sync.dma_start(trh16[k][:, c0:c1],
                                      trh16_d[k, :, c0:c1])
                    nc.sync.dma_start(trh8[:, k * COLS + c0:k * COLS + c1],
                                      trh8_d[:, k * COLS + c0:k * COLS + c1])
                    nc.sync.dma_start(trl8[:, k * COLS + c0:k * COLS + c1],
                                      trl8_d[:, k * COLS + c0:k * COLS + c1])
            nc.sync.dma_start(lab[:], lab_d[:])
            for k in range(4):
                nc.sync.dma_start(xh16[k][:, 128:B], xh16_d[k, :, 128:B])
                nc.sync.dma_start(xl8[:, k * B + 128:(k + 1) * B],
                                  xl8_d[k, :, 128:B])
                nc.sync.dma_start(xh8[:, k * B + 128:(k + 1) * B],
                                  xh8_d[k, :, 128:B])

            xl8_v = xl8[:].rearrange("p (s n) -> p s n", s=4)
            xh8_v = xh8[:].rearrange("p (s n) -> p s n", s=4)
            if DRINT:
                # interleaved pair-innermost: [p][j][n][2]
                trh8_v = trh8[:].rearrange("p (j n two) -> p j n two",
                                           j=2, two=2)
                trl8_v = trl8[:].rearrange("p (j n two) -> p j n two",
                                           j=2, two=2)
            else:
                trh8_v = trh8[:].rearrange("p (s n) -> p s n", s=4)
                trl8_v = trl8[:].rearrange("p (s n) -> p s n", s=4)

            # ------------- main loop -------------
            with tc.tile_pool(name="zbuf", bufs=2) as zbuf, \
                 tc.tile_pool(name="small", bufs=2) as small:

                def global_phase(l, ag_t):
                    gv = small.tile([128, NG], F32, name=f"gv{l}", tag="gv",
                                    bufs=1)
                    qrow = (pid_sp + 8 * l) * 128
                    for c2 in range(NCORES):
                        nc.sync.dma_start(
                            gv[:, c2 * 8:(c2 + 1) * 8],
                            ag_t[bass.ds(c2 * B + qrow, 128), 0:8])
                    t8a = small.tile([128, 8], F32, name=f"t8a{l}", tag="t8a")
                    t8b = small.tile([128, 8], F32, name=f"t8b{l}", tag="t8b")
                    gv2 = small.tile([128, NG], F32, name=f"gv2{l}", tag="gv2",
                                     bufs=1)
                    nc.vector.max(t8a[:], gv[:])
                    nc.vector.match_replace(gv2[:], t8a[:], gv[:], NEG)
                    nc.vector.max(t8b[:], gv2[:])
                    v16 = small.tile([128, K], F32, name=f"v16{l}", tag="v16")
                    nc.vector.tensor_copy(v16[:, 0:8], t8a[:])
                    nc.vector.tensor_copy(v16[:, 8:16], t8b[:])
                    labi = small.tile([128, K], I32, name=f"labi{l}",
                                      tag="labi")
                    nc.vector.tensor_scalar(
                        out=labi[:], in0=v16[:].bitcast(I32),
                        scalar1=m127[:, 0:1], scalar2=None,
                        op0=AL.bitwise_and)
                    lab16 = small.tile([128, K], F32, name=f"lab16{l}",
                                       tag="lab16")
                    nc.vector.tensor_copy(lab16[:], labi[:])
                    xn_col = small.tile([128, 1], F32, name=f"xnc{l}",
                                        tag="xncol")
                    nc.sync.dma_start(xn_col[:],
                                      xn_d[:, bass.ds(pid_sp + 8 * l, 1)])
                    dsq = small.tile([128, K], F32, name=f"dsq{l}", tag="dsq")
                    nc.scalar.activation(dsq[:], v16[:], AF.Sqrt, scale=-1.0,
                                         bias=xn_col[:, 0:1])
                    ew = small.tile([128, K], F32, name=f"ew{l}", tag="ew")
                    zsum = small.tile([128, 1], F32, name=f"zs{l}", tag="zs")
                    nc.scalar.activation(ew[:], dsq[:], AF.Exp, scale=-1.0,
                                         accum_out=zsum[:, 0:1])
                    rz = small.tile([128, 1], F32, name=f"rz{l}", tag="rz")
                    nc.vector.reciprocal(rz[:], zsum[:])
                    wt = small.tile([128, K], F32, name=f"wt{l}", tag="wt")
                    nc.vector.tensor_scalar(out=wt[:], in0=ew[:],
                                            scalar1=rz[:, 0:1], scalar2=None,
                                            op0=AL.mult)
                    vote = small.tile([128, NCLASS], F32, name=f"vote{l}",
                                      tag="vote", bufs=1)
                    tmp = small.tile([128, NCLASS], F32, name=f"vtmp{l}",
                                     tag="vtmp", bufs=1)
                    nc.vector.memset(vote[:], 0.0)
                    for r in range(K):
                        nc.vector.tensor_scalar(out=tmp[:], in0=cio_f[:],
                                                scalar1=lab16[:, r:r + 1],
                                                scalar2=wt[:, r:r + 1],
                                                op0=AL.is_equal, op1=AL.mult)
                        nc.vector.tensor_tensor(out=vote[:], in0=vote[:],
                                                in1=tmp[:], op=AL.add)
                    nc.sync.dma_start(out_d[l * 128:(l + 1) * 128, :], vote[:])

                pid_sp = nc.sync.partition_id()
                for qt in range(QTILES if STAGE >= 1 else 0):
                    qs = qt * 128
                    z = zbuf.tile([128, COLS], F32, name=f"z{qt}", tag="z")
                    for grp in (range(0, 8), range(8, 13)):
                        pss = {}
                        for c in grp:
                            pss[c] = zps.tile([128, 512], F32,
                                              name=f"ps_{qt}_{c}", tag="ps")
                        for k in range(4):
                            for c in grp:
                                cw = CHUNKS[c]
                                co = _coff(c)
                                nc.tensor.matmul(
                                    pss[c][:, :cw],
                                    xh16[k][:, qs:qs + 128],
                                    trh16[k][:, co:co + cw],
                                    start=(k == 0), stop=False)
                        if NODR:
                            for j in range(4):
                                for c in grp:
                                    cw = CHUNKS[c]
                                    co = _coff(c)
                                    xv = (xl8_v, xh8_v)[j // 2]
                                    tv = (trh8_v, trl8_v)[j // 2]
                                    for s in (2 * (j % 2), 2 * (j % 2) + 1):
                                        nc.tensor.matmul(
                                            pss[c][:, :cw],
                                            xv[:, s, qs:qs + 128],
                                            tv[:, s, co:co + cw],
                                            start=False, stop=False)
                        else:
                            for half in range(2):
                                xv = (xl8_v, xh8_v)[half]
                                tv = (trh8_v, trl8_v)[half]
                                for j in range(2):
                                    for c in grp:
                                        cw = CHUNKS[c]
                                        co = _coff(c)
                                        if DRINT:
                                            rhs = tv[:, j, co:co + cw, :]
                                        else:
                                            rhs = tv[:, 2 * j:2 * j + 2,
                                                     co:co + cw]
                                        nc.tensor.matmul(
                                            pss[c][:, :cw],
                                            xv[:, 2 * j:2 * j + 2,
                                               qs:qs + 128],
                                            rhs,
                                            start=False, stop=False,
                                            perf_mode=DR)
                        for c in grp:
                            cw = CHUNKS[c]
                            co = _coff(c)
                            nc.tensor.matmul(pss[c][:, :cw], ones2[:],
                                             ync[:, co:co + cw],
                                             start=False, stop=True)
                        for c in grp:
                            cw = CHUNKS[c]
                            co = _coff(c)
                            nc.scalar.copy(z[:, co:co + cw], pss[c][:, :cw])

                    if DEBUG and qt == 0:
                        nc.sync.dma_start(dbgz_d[:], z[:])
                    HB = 8 * 512   # group-1 column boundary (4096)
                    cand16 = small.tile([128, 16], F32, name=f"c16_{qt}",
                                        tag="cand16")
                    for h, (c0, c1) in enumerate(((0, HB), (HB, COLS))):
                        if not NOPACK:
                            zw = (z[:, c0:c1].bitcast(U16)
                                  .rearrange("p (m two) -> p m two", two=2)
                                  [:, :, 0:1])
                            nc.vector.scalar_tensor_tensor(
                                out=zw, in0=zw, scalar=m16[:, 0:1],
                                op0=AL.bitwise_and,
                                in1=lab[:, c0:c1].rearrange(
                                    "p (m one) -> p m one", one=1),
                                op1=AL.bitwise_or)
                        nc.vector.max(cand16[:, 8 * h:8 * h + 8],
                                      z[:, c0:c1])
                    cand = small.tile([128, 8], F32, name=f"cand{qt}",
                                      tag="cand")
                    nc.vector.max(cand[:], cand16[:])
                    nc.sync.dma_start(ag_in[qs:qs + 128, 0:8], cand[:])
                    if DEBUG and qt == 0:
                        nc.sync.dma_start(dbgp_d[:], z[:])
                        nc.sync.dma_start(dbgc_d[:], cand[:])

                if STAGE >= 3:
                    nc.gpsimd.collective_compute(
                        "AllGather", AL.bypass,
                        replica_groups=[list(range(NCORES))],
                        ins=[ag_in[:].opt()], outs=[ag_out[:].opt()])
                    global_phase(0, ag_out)
                    global_phase(1, ag_out)

    nc.finalize()
    return nc


_NC_CACHE = None

LDWOPT = int(os.environ.get("KNN_LDWOPT", "0"))
if LDWOPT:
    from concourse import bass_utils as _bu
    _orig_run_command = _bu.run_command

    def _patched_run_command(cmd, *a, **kw):
        if isinstance(cmd, list):
            cmd = ["--enable-ldw-opt=true" if c == "--enable-ldw-opt=false"
                   else c for c in cmd]
        return _orig_run_command(cmd, *a, **kw)

    _bu.run_command = _patched_run_command


def _host_prep(x, tf, tl):
    """Per-core host preprocessing -> list of input dicts."""
    import ml_dtypes
    bf16 = ml_dtypes.bfloat16
    e4m3 = ml_dtypes.float8_e4m3

    x = np.ascontiguousarray(np.asarray(x, dtype=np.float32))
    tf = np.ascontiguousarray(np.asarray(tf, dtype=np.float32))
    tl = np.asarray(tl)

    xs = 2.0 * x                                     # [B, D]
    xh16 = xs.astype(np.float16)
    xh16f = xh16.astype(np.float32)
    xl = xs - xh16f
    xl8 = (xl * S).astype(e4m3)
    xh8 = (xh16f / S).astype(e4m3)

    def to_k128(a):  # [B, D] -> [4, 128, B] (transposed k-split)
        return np.ascontiguousarray(a.T.reshape(4, 128, -1))

    xh16_t = to_k128(xh16)
    xl8_t = to_k128(xl8)
    xh8_t = to_k128(xh8)

    xn = (x.astype(np.float64) ** 2).sum(1).astype(np.float32) + 512.0
    xn_t = np.ascontiguousarray(xn.reshape(QTILES, 128).T)

    in_maps = []
    for c in range(NCORES):
        sl = slice(c * NSHARD, (c + 1) * NSHARD)
        labs = np.asarray(tl[sl], dtype=np.int64)
        feats = tf[sl]
        perm = np.argsort(labs, kind="stable")
        feats_s = np.ascontiguousarray(feats[perm])
        labs_s = labs[perm]

        yh16 = feats_s.astype(np.float16)
        yh16f = yh16.astype(np.float32)
        yl = feats_s - yh16f

        trh16 = np.zeros((4, 128, COLS), np.float16)
        trh16[:, :, :NSHARD] = yh16.T.reshape(4, 128, -1)
        h8 = np.zeros((4, 128, COLS), e4m3)
        l8 = np.zeros((4, 128, COLS), e4m3)
        h8[:, :, :NSHARD] = (yh16f / S).astype(e4m3).T.reshape(4, 128, -1)
        l8[:, :, :NSHARD] = (yl * S).astype(e4m3).T.reshape(4, 128, -1)
        if DRINT:
            # out[p, j, n, s] = a[2j+s, p, n] -> [128, 4*COLS]
            def inter(a):
                st = a.reshape(2, 2, 128, COLS)
                st = np.transpose(st, (2, 0, 3, 1))
                return np.ascontiguousarray(st.reshape(128, 4 * COLS))
            trh8 = inter(h8)
            trl8 = inter(l8)
        else:
            trh8 = np.ascontiguousarray(
                np.moveaxis(h8, 0, 1).reshape(128, 4 * COLS))
            trl8 = np.ascontiguousarray(
                np.moveaxis(l8, 0, 1).reshape(128, 4 * COLS))

        yn = (feats_s.astype(np.float64) ** 2).sum(1).astype(np.float32)
        t = 512.0 - yn
        y1 = t.astype(bf16).astype(np.float32)
        y2 = (t - y1).astype(bf16)
        ync = np.zeros((2, COLS), bf16)
        ync[0, :NSHARD] = y1.astype(bf16)
        ync[1, :NSHARD] = y2
        ync[0, NSHARD:] = np.float32(-30000.0)

        labp = np.zeros(COLS, np.uint16)
        labp[:NSHARD] = labs_s.astype(np.uint16)
        lab_b = np.ascontiguousarray(np.broadcast_to(labp, (128, COLS)))

        in_maps.append({
            "xh16": xh16_t, "xl8": xl8_t, "xh8": xh8_t,
            "trh16": trh16, "trh8": trh8, "trl8": trl8,
            "ync": ync, "lab": lab_b, "xn512": xn_t,
        })
    return in_maps


def kernel(x, train_features, train_labels, **run_kwargs):
    global _NC_CACHE
    in_maps = _host_prep(x, train_features, train_labels)
    if _NC_CACHE is None:
        _NC_CACHE = build()
    res = bass_utils.run_bass_kernel_spmd(
        _NC_CACHE, in_maps, core_ids=list(range(NCORES)), **run_kwargs)
    global LAST_RESULTS
    LAST_RESULTS = res
    out = np.empty((B, NCLASS), np.float32)
    for q in range(QTILES):
        l = q // NCORES
        out[q * 128:(q + 1) * 128] = (
            res.results[q % NCORES]["out"][l * 128:(l + 1) * 128])
    return out.astype(np.float32)


LAST_RESULTS = None
